# revision 12
# baseline (speedup 1.0000x reference)
"""HOSVD aggregator kernel for 8 TRN2 NeuronCores.

y[n,o] = sum_{m0..m4} G[m0,m1,m2,m3,m4] * ris0[n,m0] * ris1[n,m4]
         * ris2[n,m3] * ris3[n,m2] * U_out[m1,o],
with ris_d = X[:,d,:] @ U_stack[d].

Strategy: data-parallel over nodes (6250/core). All compute in a
"transposed" layout (features on SBUF partitions, nodes on the free dim)
so every step is a TensorE matmul or a VectorE elementwise multiply and
no on-chip transposes are needed. Host pre-packs X as bf16 with channels
on partitions, and pre-expands the factor matrices:
  A01[n,a]=ris0[n,a//8], B01[n,a]=ris1[n,a%8]  (a=(m0,m4), 64)
  z01 = A01*B01;  A23t/B23t tiled to 128 rows, z23t = A23t*B23t
  PT[n,(m1 b)] = z01 @ G2,  G2[(m0 m4),(m1,m3,m2)] = G.transpose(0,4,1,3,2)
  Q = PT * tile(z23);  y = Q @ repeat(U_out, 64, axis=0)
"""

import sys

sys.path.insert(0, "/opt/trn_rl_repo")

import os
import numpy as np
import ml_dtypes

import concourse.bass as bass
import concourse.tile as tile
from concourse import mybir
from concourse.bass_utils import run_bass_kernel_spmd

BF16 = ml_dtypes.bfloat16

N = 50000
NCORES = 8
NPC = N // NCORES            # 6250 nodes per core
T = 512                      # nodes per supertile
NSUP = NPC // T              # 12 full supertiles
TAIL = NPC - NSUP * T        # 106

# ---------------------------------------------------------------------------
# walrus rejects >1 sync wait on a Drain; Tile's tail drain carries one wait
# per logical proc. Split it into a chain of single-wait drains.
import bass_rust as _br
from concourse.vector_clock import ScopedClock as _ScopedClock


def _split_drain_and_barrier(self, tick_clock, wait_clock):
    drain_inst = self.nc.sync.drain()
    wait_clock.add_sem_waits(
        drain_inst.ins, _ScopedClock({None: tick_clock.global_clock})
    )
    si = drain_inst.ins.sync_info
    waits = list(si.on_wait)
    if len(waits) > 1:
        drain_inst.ins.sync_info = _br.SyncInfo(on_wait=waits[:1], on_update=[])
        rest = waits[1:]
        while rest:
            d2 = self.nc.sync.drain()
            chunk, rest = rest[:1], rest[1:]
            d2.ins.sync_info = _br.SyncInfo(
                on_wait=chunk, on_update=list(si.on_update) if not rest else []
            )
    self.nc.all_engine_barrier()
    assert self.sems is not None
    popped = self.nc._tile_sem_poison_stack.pop()
    assert popped is self._sem_poison
    self.nc.clear_and_free_semaphores(list(self.sems.allocated().values()))
    self.nc.all_engine_barrier()


tile.TileContext._drain_and_barrier = _split_drain_and_barrier

# Same walrus limit applies to every instruction type: peel extra sem waits
# onto single-wait NOPs emitted just before the instruction, same engine.
_SPLIT_SEQ = [0]
_orig_add_instruction = tile.TileContext._add_instruction


def _split_add_instruction(self, inst):
    si = inst.sync_info
    waits = list(si.on_wait) if si is not None else []
    if len(waits) > 1:
        for w in waits[:-1]:
            _SPLIT_SEQ[0] += 1
            nop = mybir.InstNoOp(name=f"waitsplit_{_SPLIT_SEQ[0]}", ins=[],
                                 outs=[], engine=inst.engine)
            nop.sync_info = _br.SyncInfo(on_wait=[w], on_update=[])
            _orig_add_instruction(self, nop)
        inst.sync_info = _br.SyncInfo(on_wait=[waits[-1]],
                                      on_update=list(si.on_update))
    return _orig_add_instruction(self, inst)


tile.TileContext._add_instruction = _split_add_instruction

# ---------------------------------------------------------------------------
# weight-pack free-dim offsets (all bf16, one [128, 1536] SBUF tile)
# factor weights, 8 blocks of 64 cols, contraction-chunk-major:
#   [A01c0 A23c0 B01c0 B23c0 | A01c1 A23c1 B01c1 B23c1]
# A01 = repeat(U0,8), A23 = repeat(U2,8), B01 = tile(U1,8), B23 = tile(U3,8)
_WF = 0
_G2 = 512    # [64,512] rows 0-63 : core matrix, 4 lhsT chunks of [64,128]
_UE = 1024   # 4 chunks [128,128] : U_out expanded over b
_WCOLS = 1536


def _build_nc():
    nc = bass.Bass("TRN2", target_bir_lowering=False, debug=False,
                   num_devices=NCORES)
    bf = mybir.dt.bfloat16
    f32 = mybir.dt.float32

    xm = nc.dram_tensor("xm", [NSUP, 128, 8 * T], bf, kind="ExternalInput").ap()
    xt = nc.dram_tensor("xt", [128, 8 * TAIL], bf, kind="ExternalInput").ap()
    wp = nc.dram_tensor("wp", [128, _WCOLS], bf, kind="ExternalInput").ap()
    ym = nc.dram_tensor("ym", [NSUP, 128, T], bf, kind="ExternalOutput").ap()
    yt = nc.dram_tensor("yt", [128, TAIL], bf, kind="ExternalOutput").ap()

    with tile.TileContext(nc) as tc:
        from contextlib import ExitStack
        with ExitStack() as ctx:
            wpool = ctx.enter_context(tc.tile_pool(name="w", bufs=1))
            # whole per-core input (12.8MB) fits in SBUF: prefetch every
            # supertile up front so the input stream runs at full DMA rate,
            # completely decoupled from compute.
            xpool = ctx.enter_context(tc.tile_pool(name="x", bufs=1))
            spool = ctx.enter_context(tc.tile_pool(name="s", bufs=2))
            qpool = ctx.enter_context(tc.tile_pool(name="q", bufs=2))
            # ys buffers stay pinned until their store DMA drains; during the
            # input burst the (software-queue) stores are starved of DMA
            # engines, so give them deep buffering or the WAR stalls ScalarE.
            ypool = ctx.enter_context(tc.tile_pool(name="y", bufs=6))
            pfac = ctx.enter_context(tc.tile_pool(name="pf", bufs=1, space="PSUM"))
            ppt = ctx.enter_context(tc.tile_pool(name="ppt", bufs=1, space="PSUM"))
            pyp = ctx.enter_context(tc.tile_pool(name="py", bufs=1, space="PSUM"))

            ws = wpool.tile([128, _WCOLS], bf)
            # factor weights (128KB) lead the sync queue so the first factor
            # matmuls are gated on as little data as possible; the G2/UE
            # blocks ride the second HWDGE queue (Activation) in parallel.
            nc.sync.dma_start(ws[:, 0:_G2], wp[:, 0:_G2])
            nc.scalar.dma_start(ws[:, _G2:_WCOLS], wp[:, _G2:_WCOLS])

            def wf(blk, c):
                # factor weight block blk (0=A01,1=A23,2=B01,3=B23), chunk c
                o = _WF + 256 * c + 64 * blk
                return ws[:, o:o + 64]

            # tail first: its tiny load lands immediately and warms TensorE
            # while supertile 0 is still streaming. All supertiles live in one
            # big SBUF tile (subtile deps gate each consumer on its own DMA);
            # one tile instead of 13 keeps the semaphore count - and the
            # serialized per-semaphore clears in the epilogue - small.
            order = [NSUP] + list(range(NSUP))
            n = len(order)
            xall = xpool.tile([128, 8 * (TAIL + NSUP * T)], bf, tag="xs")
            xs_list = []
            off = 0
            for s in order:
                tc_ = T if s < NSUP else TAIL
                xs = xall[:, off:off + 8 * tc_]
                off += 8 * tc_
                nc.sync.dma_start(xs, xm[s] if s < NSUP else xt[:])
                xs_list.append(xs)

            state = {}

            def emit_factor(i):
                s = order[i]
                tc_ = T if s < NSUP else TAIL
                xs = xs_list[i]

                def xc(d, c):
                    return xs[:, (2 * d + c) * tc_:(2 * d + c + 1) * tc_]

                # factor matmuls, 64-col column-pairs that stream concurrently:
                # bank1 = [A01 ; A23], bank2 = [B01 ; B23]
                psAB1 = pfac.tile([128, tc_], f32, tag="psAB1", padded_shape=[128, T])
                psAB2 = pfac.tile([128, tc_], f32, tag="psAB2", padded_shape=[128, T])
                def mm(*a, **k):
                    nc.tensor.matmul(*a, skip_group_check=True, **k)
                mm(psAB1[0:64, :], wf(0, 0), xc(0, 0), start=True, stop=False)
                mm(psAB1[64:128, :], wf(1, 0), xc(2, 0), start=True, stop=False,
                   tile_position=(0, 64))
                mm(psAB1[0:64, :], wf(0, 1), xc(0, 1), start=False, stop=True)
                mm(psAB1[64:128, :], wf(1, 1), xc(2, 1), start=False, stop=True,
                   tile_position=(0, 64))
                mm(psAB2[0:64, :], wf(2, 0), xc(1, 0), start=True, stop=False)
                mm(psAB2[64:128, :], wf(3, 0), xc(3, 0), start=True, stop=False,
                   tile_position=(0, 64))
                mm(psAB2[0:64, :], wf(2, 1), xc(1, 1), start=False, stop=True)
                mm(psAB2[64:128, :], wf(3, 1), xc(3, 1), start=False, stop=True,
                   tile_position=(0, 64))

                # one staged copy + one mul gives both pair products:
                # z[0:64] = A01*B01 = z01, z[64:128] = A23*B23 = z23
                s1 = spool.tile([128, tc_], f32, tag="s1")
                nc.scalar.copy(s1[:], psAB1[:])
                z = spool.tile([128, tc_], bf, tag="z")
                nc.vector.tensor_mul(z[:], psAB2[:], s1[:])
                # duplicate z23 to both partition halves (bf16 DVE copies run
                # packed; partition-shifted SBUF reads are fine on DVE).
                # GpSimd is far too slow for bulk copies (~19G elem/s) and a
                # concurrent second engine on the same source rows causes SBUF
                # conflicts - keep both on DVE.
                z23t = spool.tile([128, tc_], bf, tag="z23t")
                nc.vector.tensor_copy(z23t[0:64, :], z[64:128, :])
                nc.vector.tensor_copy(z23t[64:128, :], z[64:128, :])
                state[i] = [s, tc_, z, z23t, None, None]

            def emit_pt(i):
                st = state[i]
                tc_, z, z23t = st[1], st[2], st[3]
                pt01 = ppt.tile([128, 2, tc_], f32, tag="pt01", padded_shape=[128, 2, T])
                pt23 = ppt.tile([128, 2, tc_], f32, tag="pt23", padded_shape=[128, 2, T])
                for q in range(4):
                    dst = pt01[:, q, :] if q < 2 else pt23[:, q - 2, :]
                    nc.tensor.matmul(dst,
                                     ws[0:64, _G2 + 128 * q:_G2 + 128 * (q + 1)],
                                     z[0:64, :], start=True, stop=True,
                                     skip_group_check=True)
                # chunks 0-1 staged to SBUF bf16 by ScalarE (DVE then multiplies
                # at 16-bit 2x rate); chunks 2-3 multiplied straight out of
                # PSUM by DVE, emitted first so pt23 banks free earliest.
                pb01 = spool.tile([128, 2, tc_], bf, tag="pb01")
                nc.scalar.copy(pb01[:], pt01[:])
                z23b = z23t[:].rearrange("p (a f) -> p a f", a=1).broadcast_to(
                    [128, 2, tc_])
                qt23 = qpool.tile([128, 2, tc_], bf, tag="qt23")
                nc.vector.tensor_mul(qt23[:], pt23[:], z23b)
                qt01 = qpool.tile([128, 2, tc_], bf, tag="qt01")
                nc.vector.tensor_mul(qt01[:], pb01[:], z23b)
                st[4] = qt01
                st[5] = qt23

            def emit_y(i):
                s, tc_, _, _, qt01, qt23 = state.pop(i)
                psy = pyp.tile([128, tc_], f32, tag="psy", padded_shape=[128, T])
                for q in (2, 3, 0, 1):
                    src = qt01[:, q, :] if q < 2 else qt23[:, q - 2, :]
                    nc.tensor.matmul(psy[:],
                                     ws[:, _UE + 128 * q:_UE + 128 * (q + 1)],
                                     src, start=(q == 2), stop=(q == 1),
                                     skip_group_check=True)
                ys = ypool.tile([128, tc_], bf, tag="ys")
                nc.scalar.copy(ys[:], psy[:])
                yeng = nc.sync if i == n - 1 else nc.gpsimd
                yeng.dma_start(ym[s] if s < NSUP else yt[:], ys[:])

            # two-stage software pipeline on the PE queue: factor(i) issues
            # before PT(i-1) and y(i-2), so PE always has ready work while
            # the Scalar->DVE z-chain of the current tile completes.
            for i in range(n + 2):
                if i < n:
                    emit_factor(i)
                if 0 <= i - 1 < n:
                    emit_pt(i - 1)
                if 0 <= i - 2 < n:
                    emit_y(i - 2)
    return nc


def _host_pack_weights(G, U_stack, U_output):
    U = np.asarray(U_stack, np.float32)
    Uo = np.asarray(U_output, np.float32)
    Gf = np.asarray(G, np.float32)
    wpk = np.zeros((128, _WCOLS), np.float32)
    # factor blocks (order must match wf()): A01, A23, B01, B23
    blocks = [np.repeat(U[0], 8, axis=1),      # [256,64] a[m0] repeated
              np.repeat(U[2], 8, axis=1),      # [256,64] c[m3] repeated
              np.tile(U[1], (1, 8)),           # [256,64] b[m4] tiled
              np.tile(U[3], (1, 8))]           # [256,64] d[m2] tiled
    for c in range(2):
        for blk in range(4):
            o = _WF + 256 * c + 64 * blk
            wpk[:, o:o + 64] = blocks[blk][128 * c:128 * (c + 1)]
    G2 = np.ascontiguousarray(Gf.transpose(0, 4, 1, 3, 2)).reshape(64, 512)
    wpk[0:64, _G2:_G2 + 512] = G2
    Uexp = np.repeat(Uo, 64, axis=0)           # [512,128]
    for q in range(4):
        wpk[:, _UE + 128 * q:_UE + 128 * (q + 1)] = Uexp[128 * q:128 * (q + 1)]
    return wpk.astype(BF16)


def _install_ntff_hook():
    import types
    if "antenv.axon_hooks" in sys.modules:
        return
    mod = types.ModuleType("antenv.axon_hooks")
    holder = {"hook": None}
    mod.set_axon_ntff_profile_hook = lambda h: holder.__setitem__("hook", h)
    mod.get_axon_ntff_profile_hook = lambda: holder["hook"]
    sys.modules["antenv.axon_hooks"] = mod
    import antenv
    antenv.axon_hooks = mod
    from trn_agent_boot.trn_boot import _ntff_profile_via_ctypes
    mod.set_axon_ntff_profile_hook(_ntff_profile_via_ctypes("/opt/axon/libaxon_pjrt.so"))


_NC_CACHE = None


def kernel(neighbour_states, G, U_stack, U_output):
    global _NC_CACHE
    X = np.asarray(neighbour_states, np.float32)
    wpb = _host_pack_weights(G, U_stack, U_output)

    in_maps = []
    for c in range(NCORES):
        sh = X[c * NPC:(c + 1) * NPC]                      # [6250, 4, 256]
        main = (sh[:NSUP * T]
                .reshape(NSUP, T, 4, 2, 128)
                .transpose(0, 4, 2, 3, 1)                  # [s, p, d, ch, t]
                .reshape(NSUP, 128, 8 * T))
        tail = (sh[NSUP * T:]
                .reshape(TAIL, 4, 2, 128)
                .transpose(3, 1, 2, 0)
                .reshape(128, 8 * TAIL))
        in_maps.append({
            "xm": np.ascontiguousarray(main).astype(BF16),
            "xt": np.ascontiguousarray(tail).astype(BF16),
            "wp": wpb,
        })

    if _NC_CACHE is None:
        _NC_CACHE = _build_nc()
    nc = _NC_CACHE

    trace = bool(os.environ.get("HOSVD_TRACE"))
    if trace:
        _install_ntff_hook()
    res = run_bass_kernel_spmd(nc, in_maps, core_ids=list(range(NCORES)),
                               trace=trace)
    if trace and res.exec_time_ns is not None:
        print(f"HW exec time: {res.exec_time_ns} ns")

    out = np.empty((N, 128), np.float32)
    for c in range(NCORES):
        ymc = np.asarray(res.results[c]["ym"]).astype(np.float32)  # [12,128,512]
        ytc = np.asarray(res.results[c]["yt"]).astype(np.float32)  # [128,106]
        base = c * NPC
        out[base:base + NSUP * T] = ymc.transpose(0, 2, 1).reshape(NSUP * T, 128)
        out[base + NSUP * T:base + NPC] = ytc.T
    return out

